# revision 2
# baseline (speedup 1.0000x reference)
"""Trainium2 Bass kernel v2 for nn_BeansAttentionBlock (sparse attention block).

8 cores = 4 batches x 2 query-halves; each core handles 513 query tokens
(cls + 512 patches) in transposed [dim, token] layout.

Key structure vs v1:
- LayerNorm scale/bias folded into the following matmul weights on the host
  (zero-bias fast path compiled when all effective biases vanish).
- QKV projection and attention scores run as fp8(e4m3) DoubleRow matmuls
  (2 cols/cycle); AV and MLP stay bf16 for accuracy.
- Batched exp over 4-chunk score groups (2 PSUM banks / one Act instr).
- Attention / proj / LN2 / MLP processed in two query-column halves so the
  Act-heavy attention of half B overlaps the PE-heavy MLP of half A.
- Elementwise work spread across Act / DVE / Pool engines.
"""

import numpy as np
import ml_dtypes
from contextlib import ExitStack

import concourse.bass as bass
import concourse.bacc as bacc
import concourse.tile as tile
from concourse import mybir
from concourse.bass_utils import run_bass_kernel_spmd

F32 = mybir.dt.float32
BF16 = mybir.dt.bfloat16
FP8 = mybir.dt.float8e4
NPBF = ml_dtypes.bfloat16
NPF8 = ml_dtypes.float8_e4m3
AF = mybir.ActivationFunctionType
OP = mybir.AluOpType
PM = mybir.MatmulPerfMode

B, P, KNB, D, H = 4, 1024, 32, 768, 12
S = P + 1            # 1025 tokens
HD = D // H          # 64
NCH = D // 128       # 6
TOK = 513            # query cols: [512 own patches, cls]
KC = 8               # 1024 keys = 8 chunks of 128
VW = 11 * 65 + 128   # V-augmented width
SCALE = float(HD) ** -0.5

S_PAD = 1040         # fp8 token-axis stride (16B-aligned for DoubleRow)
BLK3 = [(0, 342), (342, 342), (684, 341)]     # token blocks for LN1/K
THALVES = [(0, 256), (256, 257)]              # token-col halves (B incl cls)
QHALVES = [(0, 256), (256, 256)]              # query-col halves (patches)

_NC = {}


def _patch_act_tables():
    import concourse.bacc as _b
    import concourse.hw_specs as _h
    if getattr(_h, "_act_tables_patched", False):
        return
    orig = _h.get_activation_tables

    def filtered(arch):
        t = orig(arch)
        keep = ("natural_log_exp_and_others", "gelu_and_others")
        if not all(k in t for k in keep):
            return t
        return {k: (v if k in keep else set()) for k, v in t.items()}

    _h.get_activation_tables = filtered
    _b.get_activation_tables = filtered
    _h._act_tables_patched = True


def _bcast_ap(ap_, n):
    """Partition-broadcast view of a [1, ...] AP for DMA reads."""
    return bass.AP(tensor=ap_.tensor, offset=ap_.offset,
                   ap=[[0, n]] + [list(d) for d in ap_.ap[1:]])


def _build(has_bias):
    _patch_act_tables()
    nc = bacc.Bacc("TRN2", target_bir_lowering=False, debug=False,
                   num_devices=8)

    def din(name, shape, dt=F32):
        return nc.dram_tensor(name, shape, dt, kind="ExternalInput").ap()

    T = dict(
        xb_d=din("xb", [D, S], BF16),
        xtow_d=din("xtow", [D, TOK]),
        wqkv8_d=din("wqkv8", [D, 3 * D], FP8),
        wproj_d=din("wproj", [D, D], BF16),
        w1_d=din("w1", [D, 4 * D], BF16),
        w2_d=din("w2", [4 * D, D], BF16),
        ct_d=din("ct", [P, 512], BF16),
        qkvbrow_d=din("qkvbrow", [1, 3 * D], BF16),
        b1row_d=din("b1row", [1, 4 * D], BF16),
        pb_d=din("pb", [D]),
        b2_d=din("b2", [D]),
    )
    T["out_d"] = nc.dram_tensor("out", [D, TOK], F32,
                                kind="ExternalOutput").ap()
    T["has_bias"] = has_bias

    with tile.TileContext(nc) as tc:
        _emit(nc, tc, T)
    nc.compile()
    return nc


def _emit(nc, tc, T):
    has_bias = T["has_bias"]
    dmaq = [nc.sync, nc.scalar, nc.gpsimd]
    dqi = [0]

    def dma(out, in_):
        eng = dmaq[dqi[0] % len(dmaq)]
        dqi[0] += 1
        eng.dma_start(out=out, in_=in_)

    with ExitStack() as ctx:
        g = ctx.enter_context(tc.tile_pool(name="g", bufs=1))
        big = ctx.enter_context(tc.tile_pool(name="big", bufs=1))

        # ---------------- persistent tiles ----------------
        kt8 = big.tile([128, NCH, S_PAD], FP8, tag="kt8")
        qz8 = big.tile([128, 2, H, 512], FP8, tag="qz8")
        qcb = big.tile([128, NCH, H], FP8, tag="qcb")
        vp = big.tile([128, KC, VW], BF16, tag="vp")
        vc = big.tile([1, VW], BF16, tag="vc")
        ct = big.tile([128, KC, 512], BF16, tag="ct")
        ao = big.tile([128, NCH, TOK], BF16, tag="ao")
        x2 = big.tile([128, NCH, TOK], F32, tag="x2")
        
        xtow = big.tile([128, NCH, TOK], F32, tag="xtow")
        wproj = big.tile([128, NCH, D], BF16, tag="wproj")

        with ExitStack() as c1:
            s1p = c1.enter_context(tc.tile_pool(name="s1p", bufs=1))
            xbt = s1p.tile([128, NCH, S], BF16, tag="xbt")
            u8 = s1p.tile([128, NCH, S_PAD], FP8, tag="u8")
            wqkv8 = s1p.tile([128, NCH, 3 * D], FP8, tag="wqkv8")
            rb_sb = s1p.tile([1, S], BF16, tag="rb_sb")
            mrb_sb = s1p.tile([1, S], BF16, tag="mrb_sb")
            r_bc = s1p.tile([128, S], BF16, tag="r_bc")
            mr_bc = s1p.tile([128, S], BF16, tag="mr_bc")

            # DMAs in need-order
            for c in range(NCH):
                dma(xbt[:, c, :],
                    T["xb_d"].rearrange("(c p) t -> p c t", p=128)[:, c, :])
            nc.sync.dma_start(
                out=wqkv8,
                in_=T["wqkv8_d"].rearrange("(c p) n -> p c n", p=128))
            dma(ct, T["ct_d"].rearrange("(kc p) q -> p kc q", p=128))
            dma(xtow, T["xtow_d"].rearrange("(c p) t -> p c t", p=128))
            dma(wproj, T["wproj_d"].rearrange("(c p) n -> p c n", p=128))

            ones_c = g.tile([128, 1], BF16, tag="ones_c")
            nc.vector.memset(ones_c, 1.0)
            ones_r = g.tile([1, 128], BF16, tag="ones_r")
            nc.vector.memset(ones_r, 1.0)
            ones_n = g.tile([1, 512], BF16, tag="ones_n")
            nc.vector.memset(ones_n, 1.0)
            eps_t = g.tile([128, 1], F32, tag="eps_t")
            nc.vector.memset(eps_t, 1e-5)
            if has_bias:
                qkvbrow = g.tile([1, 3 * D], BF16, tag="qkvbrow")
                dma(qkvbrow, T["qkvbrow_d"])
                b1row = g.tile([1, 4 * D], BF16, tag="b1row")
                dma(b1row, T["b1row_d"])
            else:
                qkvbrow = b1row = None

            def vec_tile(dram, n, tag):
                t = g.tile([128, n // 128], F32, tag=tag)
                dma(t, dram.rearrange("(c p) -> p c", p=128))
                return t

            pb_t = vec_tile(T["pb_d"], D, "pb_t")
            b2_t = vec_tile(T["b2_d"], D, "b2_t")

            nc.vector.memset(qz8, 0.0)
            nc.gpsimd.memset(qcb, 0.0)

            # ================ P1: LN1 (stats + u8) ====================
            with tc.tile_pool(name="p1w", bufs=2) as p1w, \
                 tc.tile_pool(name="pp1", bufs=2, space="PSUM") as pp1:
                for (o, n) in BLK3:
                    s12 = pp1.tile([65, 512], F32, tag="s12")
                    for c in range(NCH):
                        sq = p1w.tile([128, 512], BF16, tag="sq")
                        nc.scalar.activation(
                            sq[:, :n], xbt[:, c, o:o + n], AF.Square)
                        nc.tensor.matmul(
                            s12[0:1, :n], lhsT=ones_c, rhs=xbt[:, c, o:o + n],
                            start=(c == 0), stop=(c == NCH - 1))
                        nc.tensor.matmul(
                            s12[64:65, :n], lhsT=ones_c, rhs=sq[:, :n],
                            start=(c == 0), stop=(c == NCH - 1))
                    m_f = p1w.tile([1, 512], F32, tag="m_f")
                    nc.scalar.mul(m_f[:, :n], s12[0:1, :n], 1.0 / D)
                    v = p1w.tile([1, 512], F32, tag="v")
                    nc.vector.tensor_mul(v[:, :n], m_f[:, :n], m_f[:, :n])
                    nc.vector.scalar_tensor_tensor(
                        v[:, :n], s12[64:65, :n], 1.0 / D, v[:, :n],
                        OP.mult, OP.subtract)
                    nc.scalar.activation(
                        v[:, :n], v[:, :n], AF.Ln, bias=eps_t[0:1, :])
                    nc.scalar.activation(
                        rb_sb[:, o:o + n], v[:, :n], AF.Exp, scale=-0.5)
                    nc.vector.tensor_mul(
                        mrb_sb[:, o:o + n], m_f[:, :n], rb_sb[:, o:o + n])
                    nc.gpsimd.partition_broadcast(
                        r_bc[:, o:o + n], rb_sb[:, o:o + n])
                    nc.gpsimd.partition_broadcast(
                        mr_bc[:, o:o + n], mrb_sb[:, o:o + n])
                    for c in range(NCH):
                        tmp = p1w.tile([128, 512], BF16, tag="tmp")
                        nc.vector.tensor_mul(
                            tmp[:, :n], xbt[:, c, o:o + n], r_bc[:, o:o + n])
                        nc.vector.tensor_sub(
                            u8[:, c, o:o + n], tmp[:, :n], mr_bc[:, o:o + n])

            # ================ P2: QKV (fp8 DoubleRow) ==================
            KOFF, VOFF = D, 2 * D

            def dr_mm(ps, n, col, off, tok_o, stop_last=True):
                """ps[:, :n] += wqkv8[:, :, col:col+128].T @ u8[:, :, tok]"""
                for cp in range(3):
                    nc.tensor.matmul(
                        ps[:, :n],
                        lhsT=wqkv8[:, 2 * cp:2 * cp + 2, off + col:off + col + 128],
                        rhs=u8[:, 2 * cp:2 * cp + 2, tok_o:tok_o + n],
                        start=(cp == 0),
                        stop=(cp == 2 and stop_last),
                        perf_mode=PM.DoubleRow)

            def bias_mm(ps, n, col, off, width=128):
                nc.tensor.matmul(
                    ps[:, :n], lhsT=qkvbrow[:, off + col:off + col + width],
                    rhs=ones_n[0:1, :n], start=False, stop=True)

            with tc.tile_pool(name="pp2", bufs=3, space="PSUM") as pp2:
                # --- K^T: [dims, 1025 tokens], written to kt8 as fp8
                for dd in range(NCH):
                    for (o, n) in BLK3:
                        ps = pp2.tile([128, 512], F32, tag="mm")
                        dr_mm(ps, n, dd * 128, KOFF, o, stop_last=not has_bias)
                        if has_bias:
                            bias_mm(ps, n, dd * 128, KOFF)
                        nc.scalar.copy(kt8[:, dd, o:o + n], ps[:, :n])
                # --- Q: patch cols 1..512 -> qz8, cls col 0 -> qcb
                for dd in range(NCH):
                    ps = pp2.tile([128, 512], F32, tag="mm")
                    dr_mm(ps, 512, dd * 128, 0, 0, stop_last=not has_bias)
                    if has_bias:
                        bias_mm(ps, 512, dd * 128, 0)
                    psc = pp2.tile([128, 512], F32, tag="mm", name="mm")[:, 0:1]
                    dr_mm(psc, 1, dd * 128, 0, 1024, stop_last=not has_bias)
                    if has_bias:
                        bias_mm(psc, 1, dd * 128, 0)
                    for i in range(2):
                        h = 2 * dd + i
                        b0 = i * 64
                        nc.vector.tensor_copy(
                            qz8[b0:b0 + 64, 0, h, :], ps[b0:b0 + 64, :])
                        nc.scalar.copy(
                            qcb[b0:b0 + 64, dd, h:h + 1], psc[b0:b0 + 64, :])
                # --- V: natural [token, dim], 65-block layout + ones cols
                for kc in range(KC):
                    for (o, n) in [(0, 384), (384, 384)]:
                        ps = pp2.tile([128, 512], F32, tag="mm", name="mm")[:, 0:384]
                        for cp in range(3):
                            nc.tensor.matmul(
                                ps[:, :n],
                                lhsT=u8[:, 2 * cp:2 * cp + 2,
                                        kc * 128:(kc + 1) * 128],
                                rhs=wqkv8[:, 2 * cp:2 * cp + 2,
                                          VOFF + o:VOFF + o + n],
                                start=(cp == 0),
                                stop=(cp == 2 and not has_bias),
                                perf_mode=PM.DoubleRow)
                        if has_bias:
                            nc.tensor.matmul(
                                ps[:, :n], lhsT=ones_r[:, 0:128],
                                rhs=qkvbrow[:, VOFF + o:VOFF + o + n],
                                start=False, stop=True)
                        dstv = vp[:, kc, 0:780].rearrange(
                            "p (h x) -> p h x", x=65)[:, o // 64:o // 64 + 6, 0:64]
                        nc.vector.tensor_copy(
                            dstv, ps[:, :n].rearrange("p (h x) -> p h x", x=64))
                # cls V row
                for (o, n) in [(0, 384), (384, 384)]:
                    ps = pp2.tile([128, 512], F32, tag="mm", name="mm")[:, 0:384]
                    for cp in range(3):
                        nc.tensor.matmul(
                            ps[0:1, :n],
                            lhsT=u8[:, 2 * cp:2 * cp + 2, 1024:1025],
                            rhs=wqkv8[:, 2 * cp:2 * cp + 2,
                                      VOFF + o:VOFF + o + n],
                            start=(cp == 0),
                            stop=(cp == 2 and not has_bias),
                            perf_mode=PM.DoubleRow)
                    if has_bias:
                        nc.tensor.matmul(
                            ps[0:1, :n], lhsT=ones_r[:, 0:1],
                            rhs=qkvbrow[:, VOFF + o:VOFF + o + n],
                            start=False, stop=True)
                    dstv = vc[:, 0:780].rearrange(
                        "p (h x) -> p h x", x=65)[:, o // 64:o // 64 + 6, 0:64]
                    nc.scalar.copy(
                        dstv, ps[0:1, :n].rearrange("p (h x) -> p h x", x=64))
                nc.gpsimd.memset(
                    vp[:, :, 0:780].rearrange(
                        "p k (h x) -> p k h x", x=65)[:, :, :, 64:65], 1.0)
                nc.gpsimd.memset(vp[:, :, 780:], 0.0)
                nc.gpsimd.memset(
                    vc[:, 0:780].rearrange(
                        "p (h x) -> p h x", x=65)[:, :, 64:65], 1.0)
                nc.gpsimd.memset(vc[:, 780:], 0.0)

            # ================ P2.5: CLS dense attention ================
            with tc.tile_pool(name="ppc", bufs=1, space="PSUM") as ppc, \
                 tc.tile_pool(name="pcw", bufs=1) as pcw:
                pcl = ppc.tile([128, 108], F32, tag="pcl")
                for kc in range(KC):
                    for cp in range(3):
                        nc.tensor.matmul(
                            pcl[:, kc * 12:(kc + 1) * 12],
                            lhsT=kt8[:, 2 * cp:2 * cp + 2,
                                     kc * 128:(kc + 1) * 128],
                            rhs=qcb[:, 2 * cp:2 * cp + 2, :],
                            start=(cp == 0), stop=(cp == 2),
                            perf_mode=PM.DoubleRow)
                for cp in range(3):
                    nc.tensor.matmul(
                        pcl[0:1, 96:108],
                        lhsT=kt8[:, 2 * cp:2 * cp + 2, 1024:1025],
                        rhs=qcb[:, 2 * cp:2 * cp + 2, :],
                        start=(cp == 0), stop=(cp == 2),
                        perf_mode=PM.DoubleRow)
                ebs = pcw.tile([128, 108], BF16, tag="ebs")
                nc.scalar.activation(ebs[:, 0:96], pcl[:, 0:96],
                                     AF.Exp, scale=SCALE)
                nc.scalar.activation(ebs[0:1, 96:108], pcl[0:1, 96:108],
                                     AF.Exp, scale=SCALE)
                pd = ppc.tile([1, 12], F32, tag="pd")
                for kc in range(KC):
                    nc.tensor.matmul(
                        pd, lhsT=ones_c, rhs=ebs[:, kc * 12:(kc + 1) * 12],
                        start=(kc == 0), stop=False)
                nc.tensor.matmul(
                    pd, lhsT=ones_c[0:1, :], rhs=ebs[0:1, 96:108],
                    start=False, stop=True)
                rB = pcw.tile([1, 12], F32, tag="rB")
                nc.vector.reciprocal_approx_fast(rB, pd)
                rBc = pcw.tile([128, 12], F32, tag="rBc")
                nc.gpsimd.partition_broadcast(rBc, rB)
                for h in range(H):
                    poB = ppc.tile([64, 1], F32, tag="poB")
                    for kc in range(KC):
                        nc.tensor.matmul(
                            poB, lhsT=vp[:, kc, h * 65:h * 65 + 64],
                            rhs=ebs[:, kc * 12 + h:kc * 12 + h + 1],
                            start=(kc == 0), stop=False)
                    nc.tensor.matmul(
                        poB, lhsT=vc[:, h * 65:h * 65 + 64],
                        rhs=ebs[0:1, 96 + h:97 + h],
                        start=False, stop=True)
                    b0 = (h % 2) * 64
                    nc.scalar.activation(
                        ao[b0:b0 + 64, h // 2, 512:513], poB,
                        AF.Copy, scale=rBc[0:64, h:h + 1])

        # ============ P3..P6: attention + proj + LN2 + MLP =============
        with ExitStack() as c2:
            s2p = c2.enter_context(tc.tile_pool(name="s2p", bufs=1))
            w1t = s2p.tile([128, NCH, 4 * D], BF16, tag="w1t")
            dma(w1t, T["w1_d"].rearrange("(c p) n -> p c n", p=128))
            w2t = s2p.tile([128, 24, D], BF16, tag="w2t")
            dma(w2t, T["w2_d"].rearrange("(c p) n -> p c n", p=128))
            h1 = s2p.tile([128, 24, 257], BF16, tag="h1")

            psc_p = c2.enter_context(
                tc.tile_pool(name="psc", bufs=2, space="PSUM"))
            pav_p = c2.enter_context(
                tc.tile_pool(name="pav", bufs=2, space="PSUM"))
            pgen = c2.enter_context(
                tc.tile_pool(name="pgen", bufs=2, space="PSUM"))
            pet = c2.enter_context(tc.tile_pool(name="pet", bufs=2))
            pwt = c2.enter_context(tc.tile_pool(name="pwt", bufs=2))
            pns = c2.enter_context(tc.tile_pool(name="pns", bufs=2))
            pw = c2.enter_context(tc.tile_pool(name="pw", bufs=2))
            pfin = c2.enter_context(tc.tile_pool(name="pfin", bufs=2))
            xn2 = s2p.tile([128, NCH, 257], BF16, tag="xn2")

            kt8_ap = kt8

            def kpair(ch, kc):
                """[128, 2, 128] DR lhsT: sub0 = chunk ch keys, sub1 junk."""
                base = kt8_ap[:, ch, kc * 128:(kc + 1) * 128]
                step = S_PAD if ch < NCH - 1 else -S_PAD
                return bass.AP(tensor=base.tensor, offset=base.offset,
                               ap=[list(base.ap[0]), [step, 2], [1, 128]])

            def attn_half(hi):
                c0, nq = QHALVES[hi]
                pend = []

                def flush():
                    av_, rb_, ch_, b0_ = pend.pop(0)
                    nc.vector.tensor_mul(
                        ao[b0_:b0_ + 64, ch_, c0:c0 + nq], av_[0:64, :], rb_)

                for h in range(H):
                    ch, b0 = h // 2, (h % 2) * 64
                    av = pav_p.tile([128, 512], F32, tag="av",
                                    name="av")[:, 0:256]
                    for pg in range(2):
                        sc4 = psc_p.tile([128, 4, 256], F32, tag="sc4")
                        for j in range(4):
                            nc.tensor.matmul(
                                sc4[:, j, :], lhsT=kpair(ch, 4 * pg + j),
                                rhs=qz8[:, :, h, c0:c0 + nq],
                                start=True, stop=True,
                                perf_mode=PM.DoubleRow)
                        et4 = pet.tile([128, 4, 256], BF16, tag="et4")
                        nc.scalar.activation(et4, sc4, AF.Exp, scale=SCALE)
                        wt4 = pwt.tile([128, 4, 256], BF16, tag="wt4")
                        nc.vector.tensor_mul(
                            wt4, et4, ct[:, 4 * pg:4 * pg + 4, c0:c0 + nq])
                        for j in range(4):
                            nc.tensor.matmul(
                                av, lhsT=vp[:, 4 * pg + j, h * 65:h * 65 + 128],
                                rhs=wt4[:, j, :],
                                start=(pg == 0 and j == 0),
                                stop=(pg == 1 and j == 3))
                    srow = pns.tile([1, 256], F32, tag="srow")
                    nc.scalar.copy(srow, av[64:65, :])
                    rec = pns.tile([1, 256], F32, tag="rec")
                    nc.vector.reciprocal_approx_fast(rec, srow)
                    rec_bc = pns.tile([64, 256], F32, tag="rec_bc")
                    nc.gpsimd.partition_broadcast(rec_bc, rec)
                    pend.append((av, rec_bc, ch, b0))
                    if len(pend) > 1:
                        flush()
                while pend:
                    flush()

            def proj_half(hi):
                c0, nt = THALVES[hi]
                T["s12_" + str(hi)] = s12 = pav_p.tile(
                    [128, 512], F32, tag="av", name="av")
                for dd in range(NCH):
                    ps = pgen.tile([128, 512], F32, tag="gg", name="gg")
                    for c in range(NCH):
                        nc.tensor.matmul(
                            ps[:, :nt], lhsT=wproj[:, c, dd * 128:(dd + 1) * 128],
                            rhs=ao[:, c, c0:c0 + nt],
                            start=(c == 0), stop=(c == NCH - 1))
                    nc.vector.scalar_tensor_tensor(
                        x2[:, dd, c0:c0 + nt], ps[:, :nt], pb_t[:, dd:dd + 1],
                        xtow[:, dd, c0:c0 + nt], OP.add, OP.add)
                    xc = pw.tile([128, 512], BF16, tag="xc")
                    nc.scalar.copy(xc[:, :nt], x2[:, dd, c0:c0 + nt])
                    sq2 = pw.tile([128, 512], BF16, tag="sq2")
                    nc.vector.tensor_mul(sq2[:, :nt], xc[:, :nt], xc[:, :nt])
                    nc.tensor.matmul(
                        s12[0:1, :nt], lhsT=ones_c, rhs=xc[:, :nt],
                        start=(dd == 0), stop=(dd == NCH - 1))
                    nc.tensor.matmul(
                        s12[64:65, :nt], lhsT=ones_c, rhs=sq2[:, :nt],
                        start=(dd == 0), stop=(dd == NCH - 1))

            def ln2_half(hi):
                c0, nt = THALVES[hi]
                s12 = T["s12_" + str(hi)]
                m2 = pw.tile([1, 512], F32, tag="m2")
                nc.scalar.mul(m2[:, :nt], s12[0:1, :nt], 1.0 / D)
                v2 = pw.tile([1, 512], F32, tag="v2")
                nc.vector.tensor_mul(v2[:, :nt], m2[:, :nt], m2[:, :nt])
                nc.vector.scalar_tensor_tensor(
                    v2[:, :nt], s12[64:65, :nt], 1.0 / D, v2[:, :nt],
                    OP.mult, OP.subtract)
                nc.scalar.activation(
                    v2[:, :nt], v2[:, :nt], AF.Ln, bias=eps_t[0:1, :])
                r2b = pw.tile([1, 512], BF16, tag="r2b")
                nc.scalar.activation(r2b[:, :nt], v2[:, :nt], AF.Exp,
                                     scale=-0.5)
                mr2b = pw.tile([1, 512], BF16, tag="mr2b")
                nc.vector.tensor_mul(mr2b[:, :nt], m2[:, :nt], r2b[:, :nt])
                r2bc = pw.tile([128, 512], BF16, tag="r2bc")
                nc.gpsimd.partition_broadcast(r2bc[:, :nt], r2b[:, :nt])
                mr2bc = pw.tile([128, 512], BF16, tag="mr2bc")
                nc.gpsimd.partition_broadcast(mr2bc[:, :nt], mr2b[:, :nt])
                for c in range(NCH):
                    t2 = pw.tile([128, 512], BF16, tag="t2")
                    nc.vector.tensor_mul(
                        t2[:, :nt], x2[:, c, c0:c0 + nt], r2bc[:, :nt])
                    nc.vector.tensor_sub(
                        xn2[:, c, 0:nt], t2[:, :nt], mr2bc[:, :nt])

            def mlp_w1_half(hi):
                c0, nt = THALVES[hi]
                if nt == 256:
                    for dmp in range(12):
                        ps = pgen.tile([128, 512], F32, tag="gg", name="gg").rearrange("p (a b) -> p a b", a=2)
                        for i in range(2):
                            dm = 2 * dmp + i
                            for c in range(NCH):
                                nc.tensor.matmul(
                                    ps[:, i, :],
                                    lhsT=w1t[:, c, dm * 128:(dm + 1) * 128],
                                    rhs=xn2[:, c, 0:nt],
                                    start=(c == 0),
                                    stop=(c == NCH - 1 and not has_bias))
                            if has_bias:
                                nc.tensor.matmul(
                                    ps[:, i, :],
                                    lhsT=b1row[:, dm * 128:(dm + 1) * 128],
                                    rhs=ones_n[0:1, :nt],
                                    start=False, stop=True)
                        nc.scalar.activation(
                            h1[:, 2 * dmp:2 * dmp + 2, 0:nt], ps,
                            AF.Gelu)
                else:
                    for dm in range(24):
                        ps = pgen.tile([128, 512], F32, tag="gg")
                        for c in range(NCH):
                            nc.tensor.matmul(
                                ps[:, :nt],
                                lhsT=w1t[:, c, dm * 128:(dm + 1) * 128],
                                rhs=xn2[:, c, 0:nt],
                                start=(c == 0),
                                stop=(c == NCH - 1 and not has_bias))
                        if has_bias:
                            nc.tensor.matmul(
                                ps[:, :nt],
                                lhsT=b1row[:, dm * 128:(dm + 1) * 128],
                                rhs=ones_n[0:1, :nt],
                                start=False, stop=True)
                        nc.scalar.activation(
                            h1[:, dm, 0:nt], ps[:, :nt], AF.Gelu)

            def mlp_w2_half(hi):
                c0, nt = THALVES[hi]
                for dd in range(NCH):
                    ps = pgen.tile([128, 512], F32, tag="gg")
                    for cm in range(24):
                        nc.tensor.matmul(
                            ps[:, :nt],
                            lhsT=w2t[:, cm, dd * 128:(dd + 1) * 128],
                            rhs=h1[:, cm, 0:nt],
                            start=(cm == 0), stop=(cm == 23))
                    fin = pfin.tile([128, 257], F32, tag="fin")
                    nc.vector.scalar_tensor_tensor(
                        fin[:, :nt], ps[:, :nt], b2_t[:, dd:dd + 1],
                        x2[:, dd, c0:c0 + nt], OP.add, OP.add)
                    nc.sync.dma_start(
                        out=T["out_d"].rearrange(
                            "(c p) t -> p c t", p=128)[:, dd, c0:c0 + nt],
                        in_=fin[:, :nt])

            attn_half(0)
            proj_half(0)
            ln2_half(0)
            mlp_w1_half(0)
            attn_half(1)
            mlp_w2_half(0)
            proj_half(1)
            ln2_half(1)
            mlp_w1_half(1)
            mlp_w2_half(1)


def _get_nc(has_bias):
    if has_bias not in _NC:
        _NC[has_bias] = _build(has_bias)
    return _NC[has_bias]


def _host_prep(x, routes, inputs):
    f32 = np.float32
    qkv_w = np.asarray(inputs["qkv_w"], f32)
    qkv_b = np.asarray(inputs["qkv_b"], f32)
    ln1_w = np.asarray(inputs["ln1_w"], f32)
    ln1_b = np.asarray(inputs["ln1_b"], f32)
    ln2_w = np.asarray(inputs["ln2_w"], f32)
    ln2_b = np.asarray(inputs["ln2_b"], f32)
    mlp_w1 = np.asarray(inputs["mlp_w1"], f32)

    wqkv_f = ln1_w[:, None] * qkv_w
    qkvb_f = qkv_b + ln1_b @ qkv_w
    w1_f = ln2_w[:, None] * mlp_w1
    b1_f = np.asarray(inputs["mlp_b1"], f32) + ln2_b @ mlp_w1
    pb = np.asarray(inputs["proj_b"], f32)
    b2 = np.asarray(inputs["mlp_b2"], f32)

    has_bias = bool(
        np.abs(qkvb_f).max() > 0 or np.abs(b1_f).max() > 0
        or np.abs(pb).max() > 0 or np.abs(b2).max() > 0)

    shared = {
        "wqkv8": wqkv_f.astype(NPF8),
        "wproj": np.asarray(inputs["proj_w"], f32).astype(NPBF),
        "w1": w1_f.astype(NPBF),
        "w2": np.asarray(inputs["mlp_w2"], f32).astype(NPBF),
        "qkvbrow": qkvb_f.reshape(1, -1).astype(NPBF),
        "b1row": b1_f.reshape(1, -1).astype(NPBF),
        "pb": pb, "b2": b2,
    }
    r = np.asarray(routes).astype(np.int64) + 1
    in_maps, meta = [], []
    for core in range(8):
        b, gr = core // 2, core % 2
        own = np.arange(1, 513) if gr == 0 else np.arange(513, 1025)
        other = np.arange(513, 1025) if gr == 0 else np.arange(1, 513)
        tok_order = np.concatenate([own, other, [0]])
        key_of_token = np.zeros(S, np.int64)
        key_of_token[tok_order[0:P]] = np.arange(P)
        rows = key_of_token[r[own - 1]]
        C = np.zeros((P, 512), np.float32)
        np.add.at(C, (rows.ravel(), np.repeat(np.arange(512), 32)), 1)
        xT = np.ascontiguousarray(x[b][tok_order].T)          # [768, 1025]
        m = dict(shared)
        m["xb"] = xT.astype(NPBF)
        m["xtow"] = np.ascontiguousarray(
            x[b][np.concatenate([own, [0]])].T)               # [768, 513]
        m["ct"] = C.astype(NPBF)
        in_maps.append(m)
        meta.append((b, gr, own))
    return in_maps, meta, has_bias


def kernel(**inputs):
    x = np.asarray(inputs["x"], np.float32)
    routes = np.asarray(inputs["routes"])
    in_maps, meta, has_bias = _host_prep(x, routes, inputs)
    nc = _get_nc(has_bias)
    res = run_bass_kernel_spmd(nc, in_maps, list(range(8)))
    out = np.zeros((B, S, D), np.float32)
    for core in range(8):
        b, gr, own = meta[core]
        oT = np.asarray(res.results[core]["out"]).T           # [513, 768]
        out[b, own] = oT[0:512]
        if gr == 0:
            out[b, 0] = oT[512]
    return out


# revision 3
# speedup vs baseline: 1.0029x; 1.0029x over previous
"""Trainium2 Bass kernel v2 for nn_BeansAttentionBlock (sparse attention block).

8 cores = 4 batches x 2 query-halves; each core handles 513 query tokens
(cls + 512 patches) in transposed [dim, token] layout.

Key structure vs v1:
- LayerNorm scale/bias folded into the following matmul weights on the host
  (zero-bias fast path compiled when all effective biases vanish).
- QKV projection and attention scores run as fp8(e4m3) DoubleRow matmuls
  (2 cols/cycle); AV and MLP stay bf16 for accuracy.
- Batched exp over 4-chunk score groups (2 PSUM banks / one Act instr).
- Attention / proj / LN2 / MLP processed in two query-column halves so the
  Act-heavy attention of half B overlaps the PE-heavy MLP of half A.
- Elementwise work spread across Act / DVE / Pool engines.
"""

import numpy as np
import ml_dtypes
from contextlib import ExitStack

import concourse.bass as bass
import concourse.bacc as bacc
import concourse.tile as tile
from concourse import mybir
from concourse.bass_utils import run_bass_kernel_spmd

F32 = mybir.dt.float32
BF16 = mybir.dt.bfloat16
FP8 = mybir.dt.float8e4
NPBF = ml_dtypes.bfloat16
NPF8 = ml_dtypes.float8_e4m3
AF = mybir.ActivationFunctionType
OP = mybir.AluOpType
PM = mybir.MatmulPerfMode

B, P, KNB, D, H = 4, 1024, 32, 768, 12
S = P + 1            # 1025 tokens
HD = D // H          # 64
NCH = D // 128       # 6
TOK = 513            # query cols: [512 own patches, cls]
KC = 8               # 1024 keys = 8 chunks of 128
VW = 11 * 65 + 128   # V-augmented width
SCALE = float(HD) ** -0.5

S_PAD = 1040         # fp8 token-axis stride (16B-aligned for DoubleRow)
BLK3 = [(0, 342), (342, 342), (684, 341)]     # token blocks for LN1/K
THALVES = [(0, 256), (256, 257)]              # token-col halves (B incl cls)
QHALVES = [(0, 256), (256, 256)]              # query-col halves (patches)

_NC = {}


def _patch_act_tables():
    import concourse.bacc as _b
    import concourse.hw_specs as _h
    if getattr(_h, "_act_tables_patched", False):
        return
    orig = _h.get_activation_tables

    def filtered(arch):
        t = orig(arch)
        keep = ("natural_log_exp_and_others", "gelu_and_others")
        if not all(k in t for k in keep):
            return t
        return {k: (v if k in keep else set()) for k, v in t.items()}

    _h.get_activation_tables = filtered
    _b.get_activation_tables = filtered
    _h._act_tables_patched = True


def _bcast_ap(ap_, n):
    """Partition-broadcast view of a [1, ...] AP for DMA reads."""
    return bass.AP(tensor=ap_.tensor, offset=ap_.offset,
                   ap=[[0, n]] + [list(d) for d in ap_.ap[1:]])


def _build(has_bias):
    _patch_act_tables()
    nc = bacc.Bacc("TRN2", target_bir_lowering=False, debug=False,
                   num_devices=8)

    def din(name, shape, dt=F32):
        return nc.dram_tensor(name, shape, dt, kind="ExternalInput").ap()

    T = dict(
        xb_d=din("xb", [D, S], BF16),
        xtow_d=din("xtow", [D, TOK]),
        wqkv8_d=din("wqkv8", [D, 3 * D], FP8),
        wproj_d=din("wproj", [D, D], BF16),
        w1_d=din("w1", [D, 4 * D], BF16),
        w2_d=din("w2", [4 * D, D], BF16),
        ct_d=din("ct", [P, 512], BF16),
        qkvbrow_d=din("qkvbrow", [1, 3 * D], BF16),
        b1row_d=din("b1row", [1, 4 * D], BF16),
        pb_d=din("pb", [D]),
        b2_d=din("b2", [D]),
    )
    T["out_d"] = nc.dram_tensor("out", [D, TOK], F32,
                                kind="ExternalOutput").ap()
    T["has_bias"] = has_bias

    with tile.TileContext(nc) as tc:
        _emit(nc, tc, T)
    nc.compile()
    return nc


def _emit(nc, tc, T):
    has_bias = T["has_bias"]
    dmaq = [nc.sync, nc.scalar, nc.gpsimd]
    dqi = [0]

    def dma(out, in_):
        eng = dmaq[dqi[0] % len(dmaq)]
        dqi[0] += 1
        eng.dma_start(out=out, in_=in_)

    with ExitStack() as ctx:
        g = ctx.enter_context(tc.tile_pool(name="g", bufs=1))
        big = ctx.enter_context(tc.tile_pool(name="big", bufs=1))

        # ---------------- persistent tiles ----------------
        kt8 = big.tile([128, NCH, S_PAD], FP8, tag="kt8")
        qz8 = big.tile([128, 2, H, 512], FP8, tag="qz8")
        qcb = big.tile([128, NCH, H], FP8, tag="qcb")
        vp = big.tile([128, KC, VW], BF16, tag="vp")
        vc = big.tile([1, VW], BF16, tag="vc")
        ct = big.tile([128, KC, 512], BF16, tag="ct")
        ao = big.tile([128, NCH, TOK], BF16, tag="ao")
        x2 = big.tile([128, NCH, TOK], F32, tag="x2")
        
        xtow = big.tile([128, NCH, TOK], F32, tag="xtow")
        wproj = big.tile([128, NCH, D], BF16, tag="wproj")

        with ExitStack() as c1:
            s1p = c1.enter_context(tc.tile_pool(name="s1p", bufs=1))
            xbt = s1p.tile([128, NCH, S], BF16, tag="xbt")
            u8 = s1p.tile([128, NCH, S_PAD], FP8, tag="u8")
            wqkv8 = s1p.tile([128, NCH, 3 * D], FP8, tag="wqkv8")
            rb_sb = s1p.tile([1, S], BF16, tag="rb_sb")
            mrb_sb = s1p.tile([1, S], BF16, tag="mrb_sb")
            r_bc = s1p.tile([128, S], BF16, tag="r_bc")
            mr_bc = s1p.tile([128, S], BF16, tag="mr_bc")

            # DMAs in need-order
            for c in range(NCH):
                dma(xbt[:, c, :],
                    T["xb_d"].rearrange("(c p) t -> p c t", p=128)[:, c, :])
            nc.sync.dma_start(
                out=wqkv8,
                in_=T["wqkv8_d"].rearrange("(c p) n -> p c n", p=128))
            dma(ct, T["ct_d"].rearrange("(kc p) q -> p kc q", p=128))
            dma(xtow, T["xtow_d"].rearrange("(c p) t -> p c t", p=128))
            dma(wproj, T["wproj_d"].rearrange("(c p) n -> p c n", p=128))

            ones_c = g.tile([128, 1], BF16, tag="ones_c")
            nc.vector.memset(ones_c, 1.0)
            ones_r = g.tile([1, 128], BF16, tag="ones_r")
            nc.vector.memset(ones_r, 1.0)
            ones_n = g.tile([1, 512], BF16, tag="ones_n")
            nc.vector.memset(ones_n, 1.0)
            eps_t = g.tile([128, 1], F32, tag="eps_t")
            nc.vector.memset(eps_t, 1e-5)
            if has_bias:
                qkvbrow = g.tile([1, 3 * D], BF16, tag="qkvbrow")
                dma(qkvbrow, T["qkvbrow_d"])
                b1row = g.tile([1, 4 * D], BF16, tag="b1row")
                dma(b1row, T["b1row_d"])
            else:
                qkvbrow = b1row = None

            def vec_tile(dram, n, tag):
                t = g.tile([128, n // 128], F32, tag=tag)
                dma(t, dram.rearrange("(c p) -> p c", p=128))
                return t

            pb_t = vec_tile(T["pb_d"], D, "pb_t")
            b2_t = vec_tile(T["b2_d"], D, "b2_t")

            nc.vector.memset(qz8, 0.0)
            nc.gpsimd.memset(qcb, 0.0)

            # ================ P1: LN1 (stats + u8) ====================
            with tc.tile_pool(name="p1w", bufs=2) as p1w, \
                 tc.tile_pool(name="pp1", bufs=2, space="PSUM") as pp1:
                for (o, n) in BLK3:
                    s12 = pp1.tile([65, 512], F32, tag="s12")
                    for c in range(NCH):
                        sq = p1w.tile([128, 512], BF16, tag="sq")
                        nc.scalar.activation(
                            sq[:, :n], xbt[:, c, o:o + n], AF.Square)
                        nc.tensor.matmul(
                            s12[0:1, :n], lhsT=ones_c, rhs=xbt[:, c, o:o + n],
                            start=(c == 0), stop=(c == NCH - 1))
                        nc.tensor.matmul(
                            s12[64:65, :n], lhsT=ones_c, rhs=sq[:, :n],
                            start=(c == 0), stop=(c == NCH - 1))
                    m_f = p1w.tile([1, 512], F32, tag="m_f")
                    nc.scalar.mul(m_f[:, :n], s12[0:1, :n], 1.0 / D)
                    v = p1w.tile([1, 512], F32, tag="v")
                    nc.vector.tensor_mul(v[:, :n], m_f[:, :n], m_f[:, :n])
                    nc.vector.scalar_tensor_tensor(
                        v[:, :n], s12[64:65, :n], 1.0 / D, v[:, :n],
                        OP.mult, OP.subtract)
                    nc.scalar.activation(
                        v[:, :n], v[:, :n], AF.Ln, bias=eps_t[0:1, :])
                    nc.scalar.activation(
                        rb_sb[:, o:o + n], v[:, :n], AF.Exp, scale=-0.5)
                    nc.vector.tensor_mul(
                        mrb_sb[:, o:o + n], m_f[:, :n], rb_sb[:, o:o + n])
                    nc.gpsimd.partition_broadcast(
                        r_bc[:, o:o + n], rb_sb[:, o:o + n])
                    nc.gpsimd.partition_broadcast(
                        mr_bc[:, o:o + n], mrb_sb[:, o:o + n])
                    for c in range(NCH):
                        tmp = p1w.tile([128, 512], BF16, tag="tmp")
                        nc.vector.tensor_mul(
                            tmp[:, :n], xbt[:, c, o:o + n], r_bc[:, o:o + n])
                        nc.vector.tensor_sub(
                            u8[:, c, o:o + n], tmp[:, :n], mr_bc[:, o:o + n])

            # ================ P2: QKV (fp8 DoubleRow) ==================
            KOFF, VOFF = D, 2 * D

            def dr_mm(ps, n, col, off, tok_o, stop_last=True):
                """ps[:, :n] += wqkv8[:, :, col:col+128].T @ u8[:, :, tok]"""
                for cp in range(3):
                    nc.tensor.matmul(
                        ps[:, :n],
                        lhsT=wqkv8[:, 2 * cp:2 * cp + 2, off + col:off + col + 128],
                        rhs=u8[:, 2 * cp:2 * cp + 2, tok_o:tok_o + n],
                        start=(cp == 0),
                        stop=(cp == 2 and stop_last),
                        perf_mode=PM.DoubleRow)

            def bias_mm(ps, n, col, off, width=128):
                nc.tensor.matmul(
                    ps[:, :n], lhsT=qkvbrow[:, off + col:off + col + width],
                    rhs=ones_n[0:1, :n], start=False, stop=True)

            with tc.tile_pool(name="pp2", bufs=3, space="PSUM") as pp2:
                # --- K^T: [dims, 1025 tokens], written to kt8 as fp8
                for dd in range(NCH):
                    for (o, n) in BLK3:
                        ps = pp2.tile([128, 512], F32, tag="mm")
                        dr_mm(ps, n, dd * 128, KOFF, o, stop_last=not has_bias)
                        if has_bias:
                            bias_mm(ps, n, dd * 128, KOFF)
                        nc.scalar.copy(kt8[:, dd, o:o + n], ps[:, :n])
                # --- Q: patch cols 1..512 -> qz8, cls col 0 -> qcb
                for dd in range(NCH):
                    ps = pp2.tile([128, 512], F32, tag="mm")
                    dr_mm(ps, 512, dd * 128, 0, 0, stop_last=not has_bias)
                    if has_bias:
                        bias_mm(ps, 512, dd * 128, 0)
                    psc = pp2.tile([128, 512], F32, tag="mm", name="mm")[:, 0:1]
                    dr_mm(psc, 1, dd * 128, 0, 1024, stop_last=not has_bias)
                    if has_bias:
                        bias_mm(psc, 1, dd * 128, 0)
                    for i in range(2):
                        h = 2 * dd + i
                        b0 = i * 64
                        nc.vector.tensor_copy(
                            qz8[b0:b0 + 64, 0, h, :], ps[b0:b0 + 64, :])
                        nc.scalar.copy(
                            qcb[b0:b0 + 64, dd, h:h + 1], psc[b0:b0 + 64, :])
                # --- V: natural [token, dim], 65-block layout + ones cols
                for kc in range(KC):
                    for (o, n) in [(0, 384), (384, 384)]:
                        ps = pp2.tile([128, 512], F32, tag="mm", name="mm")[:, 0:384]
                        for cp in range(3):
                            nc.tensor.matmul(
                                ps[:, :n],
                                lhsT=u8[:, 2 * cp:2 * cp + 2,
                                        kc * 128:(kc + 1) * 128],
                                rhs=wqkv8[:, 2 * cp:2 * cp + 2,
                                          VOFF + o:VOFF + o + n],
                                start=(cp == 0),
                                stop=(cp == 2 and not has_bias),
                                perf_mode=PM.DoubleRow)
                        if has_bias:
                            nc.tensor.matmul(
                                ps[:, :n], lhsT=ones_r[:, 0:128],
                                rhs=qkvbrow[:, VOFF + o:VOFF + o + n],
                                start=False, stop=True)
                        dstv = vp[:, kc, 0:780].rearrange(
                            "p (h x) -> p h x", x=65)[:, o // 64:o // 64 + 6, 0:64]
                        nc.vector.tensor_copy(
                            dstv, ps[:, :n].rearrange("p (h x) -> p h x", x=64))
                # cls V row
                for (o, n) in [(0, 384), (384, 384)]:
                    ps = pp2.tile([128, 512], F32, tag="mm", name="mm")[:, 0:384]
                    for cp in range(3):
                        nc.tensor.matmul(
                            ps[0:1, :n],
                            lhsT=u8[:, 2 * cp:2 * cp + 2, 1024:1025],
                            rhs=wqkv8[:, 2 * cp:2 * cp + 2,
                                      VOFF + o:VOFF + o + n],
                            start=(cp == 0),
                            stop=(cp == 2 and not has_bias),
                            perf_mode=PM.DoubleRow)
                    if has_bias:
                        nc.tensor.matmul(
                            ps[0:1, :n], lhsT=ones_r[:, 0:1],
                            rhs=qkvbrow[:, VOFF + o:VOFF + o + n],
                            start=False, stop=True)
                    dstv = vc[:, 0:780].rearrange(
                        "p (h x) -> p h x", x=65)[:, o // 64:o // 64 + 6, 0:64]
                    nc.scalar.copy(
                        dstv, ps[0:1, :n].rearrange("p (h x) -> p h x", x=64))
                nc.gpsimd.memset(
                    vp[:, :, 0:780].rearrange(
                        "p k (h x) -> p k h x", x=65)[:, :, :, 64:65], 1.0)
                nc.gpsimd.memset(vp[:, :, 780:], 0.0)
                nc.gpsimd.memset(
                    vc[:, 0:780].rearrange(
                        "p (h x) -> p h x", x=65)[:, :, 64:65], 1.0)
                nc.gpsimd.memset(vc[:, 780:], 0.0)

            # ================ P2.5: CLS dense attention ================
            with tc.tile_pool(name="ppc", bufs=1, space="PSUM") as ppc, \
                 tc.tile_pool(name="pcw", bufs=1) as pcw:
                pcl = ppc.tile([128, 108], F32, tag="pcl")
                for kc in range(KC):
                    for cp in range(3):
                        nc.tensor.matmul(
                            pcl[:, kc * 12:(kc + 1) * 12],
                            lhsT=kt8[:, 2 * cp:2 * cp + 2,
                                     kc * 128:(kc + 1) * 128],
                            rhs=qcb[:, 2 * cp:2 * cp + 2, :],
                            start=(cp == 0), stop=(cp == 2),
                            perf_mode=PM.DoubleRow)
                for cp in range(3):
                    nc.tensor.matmul(
                        pcl[0:1, 96:108],
                        lhsT=kt8[:, 2 * cp:2 * cp + 2, 1024:1025],
                        rhs=qcb[:, 2 * cp:2 * cp + 2, :],
                        start=(cp == 0), stop=(cp == 2),
                        perf_mode=PM.DoubleRow)
                ebs = pcw.tile([128, 108], BF16, tag="ebs")
                nc.scalar.activation(ebs[:, 0:96], pcl[:, 0:96],
                                     AF.Exp, scale=SCALE)
                nc.scalar.activation(ebs[0:1, 96:108], pcl[0:1, 96:108],
                                     AF.Exp, scale=SCALE)
                pd = ppc.tile([1, 12], F32, tag="pd")
                for kc in range(KC):
                    nc.tensor.matmul(
                        pd, lhsT=ones_c, rhs=ebs[:, kc * 12:(kc + 1) * 12],
                        start=(kc == 0), stop=False)
                nc.tensor.matmul(
                    pd, lhsT=ones_c[0:1, :], rhs=ebs[0:1, 96:108],
                    start=False, stop=True)
                rB = pcw.tile([1, 12], F32, tag="rB")
                nc.vector.reciprocal_approx_fast(rB, pd)
                rBc = pcw.tile([128, 12], F32, tag="rBc")
                nc.gpsimd.partition_broadcast(rBc, rB)
                for h in range(H):
                    poB = ppc.tile([64, 1], F32, tag="poB")
                    for kc in range(KC):
                        nc.tensor.matmul(
                            poB, lhsT=vp[:, kc, h * 65:h * 65 + 64],
                            rhs=ebs[:, kc * 12 + h:kc * 12 + h + 1],
                            start=(kc == 0), stop=False)
                    nc.tensor.matmul(
                        poB, lhsT=vc[:, h * 65:h * 65 + 64],
                        rhs=ebs[0:1, 96 + h:97 + h],
                        start=False, stop=True)
                    b0 = (h % 2) * 64
                    nc.scalar.activation(
                        ao[b0:b0 + 64, h // 2, 512:513], poB,
                        AF.Copy, scale=rBc[0:64, h:h + 1])

        # ============ P3..P6: attention + proj + LN2 + MLP =============
        with ExitStack() as c2:
            s2p = c2.enter_context(tc.tile_pool(name="s2p", bufs=1))
            w1t = s2p.tile([128, NCH, 4 * D], BF16, tag="w1t")
            dma(w1t, T["w1_d"].rearrange("(c p) n -> p c n", p=128))
            w2t = s2p.tile([128, 24, D], BF16, tag="w2t")
            dma(w2t, T["w2_d"].rearrange("(c p) n -> p c n", p=128))
            h1 = s2p.tile([128, 24, 257], BF16, tag="h1")

            psc_p = c2.enter_context(
                tc.tile_pool(name="psc", bufs=2, space="PSUM"))
            pav_p = c2.enter_context(
                tc.tile_pool(name="pav", bufs=2, space="PSUM"))
            pgen = c2.enter_context(
                tc.tile_pool(name="pgen", bufs=2, space="PSUM"))
            pet = c2.enter_context(tc.tile_pool(name="pet", bufs=2))
            pwt = c2.enter_context(tc.tile_pool(name="pwt", bufs=2))
            pns = c2.enter_context(tc.tile_pool(name="pns", bufs=2))
            pw = c2.enter_context(tc.tile_pool(name="pw", bufs=2))
            pfin = c2.enter_context(tc.tile_pool(name="pfin", bufs=2))
            xn2 = s2p.tile([128, NCH, 257], BF16, tag="xn2")

            kt8_ap = kt8

            def kpair(ch, kc):
                """[128, 2, 128] DR lhsT: sub0 = chunk ch keys, sub1 junk."""
                base = kt8_ap[:, ch, kc * 128:(kc + 1) * 128]
                step = S_PAD if ch < NCH - 1 else -S_PAD
                return bass.AP(tensor=base.tensor, offset=base.offset,
                               ap=[list(base.ap[0]), [step, 2], [1, 128]])

            def attn_half(hi):
                c0, nq = QHALVES[hi]
                pend = []

                def flush():
                    av_, rb_, ch_, b0_ = pend.pop(0)
                    nc.vector.tensor_mul(
                        ao[b0_:b0_ + 64, ch_, c0:c0 + nq], av_[0:64, :], rb_)

                for h in range(H):
                    ch, b0 = h // 2, (h % 2) * 64
                    av = pav_p.tile([128, 512], F32, tag="av",
                                    name="av")[:, 0:256]
                    for pg in range(2):
                        sc4 = psc_p.tile([128, 4, 256], F32, tag="sc4")
                        for j in range(4):
                            nc.tensor.matmul(
                                sc4[:, j, :], lhsT=kpair(ch, 4 * pg + j),
                                rhs=qz8[:, :, h, c0:c0 + nq],
                                start=True, stop=True,
                                perf_mode=PM.DoubleRow)
                        et4 = pet.tile([128, 4, 256], BF16, tag="et4")
                        nc.scalar.activation(et4, sc4, AF.Exp, scale=SCALE)
                        wt4 = pwt.tile([128, 4, 256], BF16, tag="wt4")
                        nc.vector.tensor_mul(
                            wt4, et4, ct[:, 4 * pg:4 * pg + 4, c0:c0 + nq])
                        for j in range(4):
                            nc.tensor.matmul(
                                av, lhsT=vp[:, 4 * pg + j, h * 65:h * 65 + 128],
                                rhs=wt4[:, j, :],
                                start=(pg == 0 and j == 0),
                                stop=(pg == 1 and j == 3))
                    srow = pns.tile([1, 256], F32, tag="srow")
                    nc.scalar.copy(srow, av[64:65, :])
                    rec = pns.tile([1, 256], F32, tag="rec")
                    nc.vector.reciprocal_approx_fast(rec, srow)
                    rec_bc = pns.tile([64, 256], F32, tag="rec_bc")
                    nc.gpsimd.partition_broadcast(rec_bc, rec)
                    pend.append((av, rec_bc, ch, b0))
                    if len(pend) > 1:
                        flush()
                while pend:
                    flush()

            def proj_half(hi):
                c0, nt = THALVES[hi]
                T["s12_" + str(hi)] = s12 = pav_p.tile(
                    [128, 512], F32, tag="av", name="av")
                for dd in range(NCH):
                    ps = pgen.tile([128, 512], F32, tag="gg", name="gg")
                    for c in range(NCH):
                        nc.tensor.matmul(
                            ps[:, :nt], lhsT=wproj[:, c, dd * 128:(dd + 1) * 128],
                            rhs=ao[:, c, c0:c0 + nt],
                            start=(c == 0), stop=(c == NCH - 1))
                    nc.vector.scalar_tensor_tensor(
                        x2[:, dd, c0:c0 + nt], ps[:, :nt], pb_t[:, dd:dd + 1],
                        xtow[:, dd, c0:c0 + nt], OP.add, OP.add)
                    xc = pw.tile([128, 512], BF16, tag="xc")
                    nc.scalar.copy(xc[:, :nt], x2[:, dd, c0:c0 + nt])
                    sq2 = pw.tile([128, 512], BF16, tag="sq2")
                    nc.vector.tensor_mul(sq2[:, :nt], xc[:, :nt], xc[:, :nt])
                    nc.tensor.matmul(
                        s12[0:1, :nt], lhsT=ones_c, rhs=xc[:, :nt],
                        start=(dd == 0), stop=(dd == NCH - 1))
                    nc.tensor.matmul(
                        s12[64:65, :nt], lhsT=ones_c, rhs=sq2[:, :nt],
                        start=(dd == 0), stop=(dd == NCH - 1))

            def ln2_half(hi):
                c0, nt = THALVES[hi]
                s12 = T["s12_" + str(hi)]
                m2 = pw.tile([1, 512], F32, tag="m2")
                nc.scalar.mul(m2[:, :nt], s12[0:1, :nt], 1.0 / D)
                v2 = pw.tile([1, 512], F32, tag="v2")
                nc.vector.tensor_mul(v2[:, :nt], m2[:, :nt], m2[:, :nt])
                nc.vector.scalar_tensor_tensor(
                    v2[:, :nt], s12[64:65, :nt], 1.0 / D, v2[:, :nt],
                    OP.mult, OP.subtract)
                nc.scalar.activation(
                    v2[:, :nt], v2[:, :nt], AF.Ln, bias=eps_t[0:1, :])
                r2b = pw.tile([1, 512], BF16, tag="r2b")
                nc.scalar.activation(r2b[:, :nt], v2[:, :nt], AF.Exp,
                                     scale=-0.5)
                mr2b = pw.tile([1, 512], BF16, tag="mr2b")
                nc.vector.tensor_mul(mr2b[:, :nt], m2[:, :nt], r2b[:, :nt])
                r2bc = pw.tile([128, 512], BF16, tag="r2bc")
                nc.gpsimd.partition_broadcast(r2bc[:, :nt], r2b[:, :nt])
                mr2bc = pw.tile([128, 512], BF16, tag="mr2bc")
                nc.gpsimd.partition_broadcast(mr2bc[:, :nt], mr2b[:, :nt])
                for c in range(NCH):
                    t2 = pw.tile([128, 512], BF16, tag="t2")
                    nc.vector.tensor_mul(
                        t2[:, :nt], x2[:, c, c0:c0 + nt], r2bc[:, :nt])
                    nc.vector.tensor_sub(
                        xn2[:, c, 0:nt], t2[:, :nt], mr2bc[:, :nt])

            def mlp_w1_half(hi):
                c0, nt = THALVES[hi]
                if nt == 256:
                    for dmp in range(12):
                        ps = pgen.tile([128, 512], F32, tag="gg", name="gg").rearrange("p (a b) -> p a b", a=2)
                        for i in range(2):
                            dm = 2 * dmp + i
                            for c in range(NCH):
                                nc.tensor.matmul(
                                    ps[:, i, :],
                                    lhsT=w1t[:, c, dm * 128:(dm + 1) * 128],
                                    rhs=xn2[:, c, 0:nt],
                                    start=(c == 0),
                                    stop=(c == NCH - 1 and not has_bias))
                            if has_bias:
                                nc.tensor.matmul(
                                    ps[:, i, :],
                                    lhsT=b1row[:, dm * 128:(dm + 1) * 128],
                                    rhs=ones_n[0:1, :nt],
                                    start=False, stop=True)
                        nc.scalar.activation(
                            h1[:, 2 * dmp:2 * dmp + 2, 0:nt], ps,
                            AF.Gelu)
                else:
                    for dm in range(24):
                        ps = pgen.tile([128, 512], F32, tag="gg")
                        for c in range(NCH):
                            nc.tensor.matmul(
                                ps[:, :nt],
                                lhsT=w1t[:, c, dm * 128:(dm + 1) * 128],
                                rhs=xn2[:, c, 0:nt],
                                start=(c == 0),
                                stop=(c == NCH - 1 and not has_bias))
                        if has_bias:
                            nc.tensor.matmul(
                                ps[:, :nt],
                                lhsT=b1row[:, dm * 128:(dm + 1) * 128],
                                rhs=ones_n[0:1, :nt],
                                start=False, stop=True)
                        nc.scalar.activation(
                            h1[:, dm, 0:nt], ps[:, :nt], AF.Gelu)

            def mlp_w2_half(hi):
                c0, nt = THALVES[hi]
                for dd in range(NCH):
                    ps = pgen.tile([128, 512], F32, tag="gg")
                    for cm in range(24):
                        nc.tensor.matmul(
                            ps[:, :nt],
                            lhsT=w2t[:, cm, dd * 128:(dd + 1) * 128],
                            rhs=h1[:, cm, 0:nt],
                            start=(cm == 0), stop=(cm == 23))
                    fin = pfin.tile([128, 257], F32, tag="fin")
                    nc.vector.scalar_tensor_tensor(
                        fin[:, :nt], ps[:, :nt], b2_t[:, dd:dd + 1],
                        x2[:, dd, c0:c0 + nt], OP.add, OP.add)
                    nc.sync.dma_start(
                        out=T["out_d"].rearrange(
                            "(c p) t -> p c t", p=128)[:, dd, c0:c0 + nt],
                        in_=fin[:, :nt])

            attn_half(0)
            proj_half(0)
            ln2_half(0)
            attn_half(1)
            mlp_w1_half(0)
            mlp_w2_half(0)
            proj_half(1)
            ln2_half(1)
            mlp_w1_half(1)
            mlp_w2_half(1)


def _get_nc(has_bias):
    if has_bias not in _NC:
        _NC[has_bias] = _build(has_bias)
    return _NC[has_bias]


def _host_prep(x, routes, inputs):
    f32 = np.float32
    qkv_w = np.asarray(inputs["qkv_w"], f32)
    qkv_b = np.asarray(inputs["qkv_b"], f32)
    ln1_w = np.asarray(inputs["ln1_w"], f32)
    ln1_b = np.asarray(inputs["ln1_b"], f32)
    ln2_w = np.asarray(inputs["ln2_w"], f32)
    ln2_b = np.asarray(inputs["ln2_b"], f32)
    mlp_w1 = np.asarray(inputs["mlp_w1"], f32)

    wqkv_f = ln1_w[:, None] * qkv_w
    qkvb_f = qkv_b + ln1_b @ qkv_w
    w1_f = ln2_w[:, None] * mlp_w1
    b1_f = np.asarray(inputs["mlp_b1"], f32) + ln2_b @ mlp_w1
    pb = np.asarray(inputs["proj_b"], f32)
    b2 = np.asarray(inputs["mlp_b2"], f32)

    has_bias = bool(
        np.abs(qkvb_f).max() > 0 or np.abs(b1_f).max() > 0
        or np.abs(pb).max() > 0 or np.abs(b2).max() > 0)

    shared = {
        "wqkv8": wqkv_f.astype(NPF8),
        "wproj": np.asarray(inputs["proj_w"], f32).astype(NPBF),
        "w1": w1_f.astype(NPBF),
        "w2": np.asarray(inputs["mlp_w2"], f32).astype(NPBF),
        "qkvbrow": qkvb_f.reshape(1, -1).astype(NPBF),
        "b1row": b1_f.reshape(1, -1).astype(NPBF),
        "pb": pb, "b2": b2,
    }
    r = np.asarray(routes).astype(np.int64) + 1
    in_maps, meta = [], []
    for core in range(8):
        b, gr = core // 2, core % 2
        own = np.arange(1, 513) if gr == 0 else np.arange(513, 1025)
        other = np.arange(513, 1025) if gr == 0 else np.arange(1, 513)
        tok_order = np.concatenate([own, other, [0]])
        key_of_token = np.zeros(S, np.int64)
        key_of_token[tok_order[0:P]] = np.arange(P)
        rows = key_of_token[r[own - 1]]
        C = np.zeros((P, 512), np.float32)
        np.add.at(C, (rows.ravel(), np.repeat(np.arange(512), 32)), 1)
        xT = np.ascontiguousarray(x[b][tok_order].T)          # [768, 1025]
        m = dict(shared)
        m["xb"] = xT.astype(NPBF)
        m["xtow"] = np.ascontiguousarray(
            x[b][np.concatenate([own, [0]])].T)               # [768, 513]
        m["ct"] = C.astype(NPBF)
        in_maps.append(m)
        meta.append((b, gr, own))
    return in_maps, meta, has_bias


def kernel(**inputs):
    x = np.asarray(inputs["x"], np.float32)
    routes = np.asarray(inputs["routes"])
    in_maps, meta, has_bias = _host_prep(x, routes, inputs)
    nc = _get_nc(has_bias)
    res = run_bass_kernel_spmd(nc, in_maps, list(range(8)))
    out = np.zeros((B, S, D), np.float32)
    for core in range(8):
        b, gr, own = meta[core]
        oT = np.asarray(res.results[core]["out"]).T           # [513, 768]
        out[b, own] = oT[0:512]
        if gr == 0:
            out[b, 0] = oT[512]
    return out
